# revision 9
# baseline (speedup 1.0000x reference)
"""DSMIL pooling kernel for 8 Trainium2 NeuronCores.

Sharding: B=4 bags x 2-way sequence split of N=16384 -> 8 shards of
[8192, 1024]. Launch 1 computes h^T = we^T @ x^T (+be) per shard on
device (memory-bound patch-embed matmul, f32r). The tiny glue (argmax
candidate selection, q/v projections: ~0.3% of FLOPs) runs on host.
Launch 2 computes attention scores s = h @ (wk@q)/sqrt(E), a local
softmax, and the attention-weighted sum U = sum_n w_n h_n on device.
Host merges the two half-shards per bag (online-softmax combine) and
applies the final [1024,2] head.
"""

import numpy as np

import concourse.mybir as mybir
import concourse.tile as tile
from concourse import bacc
from concourse.bass import ts
from concourse.bass_utils import run_bass_kernel_spmd

# ---- tile-tail drain workaround (this walrus build rejects >1 sync-wait
# per instruction on the kernel-tail Drain) ----
from concourse.vector_clock import ScopedClock

_MAX_WAITS = 1


def _patched_drain_and_barrier(self, tick_clock, wait_clock):
    probe = self.nc.sync.nop(nofuse=True, hint="tile_drain_waits")
    wait_clock.add_sem_waits(probe.ins, ScopedClock({None: tick_clock.global_clock}))
    si = probe.ins.sync_info
    waits = list(si.on_wait) if si is not None and si.on_wait else []
    if len(waits) > _MAX_WAITS:
        si.on_wait = waits[:_MAX_WAITS]
        rest = waits[_MAX_WAITS:]
        for k in range(0, len(rest), _MAX_WAITS):
            extra = self.nc.sync.nop(nofuse=True, hint="tile_drain_waits")
            esi = extra.ins.sync_info
            if esi is None:
                extra.ins.sync_info = mybir.SyncInfo(
                    on_wait=rest[k : k + _MAX_WAITS], on_update=[]
                )
            else:
                esi.on_wait = rest[k : k + _MAX_WAITS]
    self.nc.sync.drain()
    self.nc.all_engine_barrier()
    popped = self.nc._tile_sem_poison_stack.pop()
    assert popped is self._sem_poison
    self.nc.clear_and_free_semaphores(list(self.sems.allocated().values()))
    self.nc.all_engine_barrier()


tile.TileContext._drain_and_barrier = _patched_drain_and_barrier

F32 = mybir.dt.float32
F32R = mybir.dt.float32r

B, N, D, E, C = 4, 16384, 1024, 512, 2
NCORES = 8
NS = N // 2          # per-core sequence shard
NT = 512             # n-tile (psum free dim)
NTILES = NS // NT    # 16
DB = D // 128        # 8
EB = E // 128        # 4

_cache = {}


def _build_launch1():
    nc = bacc.Bacc(None, target_bir_lowering=False)
    xt_d = nc.dram_tensor("xt", [D, NS], F32R, kind="ExternalInput")
    we_d = nc.dram_tensor("we", [D, E], F32R, kind="ExternalInput")
    bec_d = nc.dram_tensor("bec", [128, EB], F32, kind="ExternalInput")
    hT_d = nc.dram_tensor("hT", [128, EB, NS], F32, kind="ExternalOutput")

    with tile.TileContext(nc) as tc:
        with (
            tc.tile_pool(name="wpool", bufs=1) as wp,
            tc.tile_pool(name="xpool", bufs=2) as xp,
            tc.tile_pool(name="hpool", bufs=3) as hp,
            tc.tile_pool(name="psum", bufs=1, space="PSUM") as pp,
        ):
            we_sb = wp.tile([128, DB, E], F32R)
            nc.sync.dma_start(we_sb[:], we_d.rearrange("(db p) e -> p db e", p=128))
            bec_sb = wp.tile([128, EB], F32)
            nc.sync.dma_start(bec_sb[:], bec_d[:])

            xt3 = xt_d.rearrange("(db p) n -> p db n", p=128)

            for ntp in range(NTILES // 2):
                x_ts = []
                for k in range(2):
                    nt = 2 * ntp + k
                    x_t = xp.tile([128, DB, NT], F32R, tag=f"xt{k}", name=f"xt{k}")
                    nc.sync.dma_start(x_t[:], xt3[:, :, ts(nt, NT)])
                    x_ts.append(x_t)
                # one weight load serves both n-tiles: loop (eb, db) outer,
                # the two matmuls with identical lhsT run back-to-back
                ps = [
                    [pp.tile([128, NT], F32, tag=f"ph{eb}_{k}", name=f"ph{eb}_{k}") for k in range(2)]
                    for eb in range(EB)
                ]
                for eb in range(EB):
                    for db in range(DB):
                        for k in range(2):
                            nc.tensor.matmul(
                                ps[eb][k][:],
                                lhsT=we_sb[:, db, ts(eb, 128)],
                                rhs=x_ts[k][:, db, :],
                                start=(db == 0),
                                stop=(db == DB - 1),
                            )
                for k in range(2):
                    nt = 2 * ntp + k
                    stg = hp.tile([128, EB, NT], F32, tag="hstg", name="hstg")
                    for eb in range(EB):
                        dst = stg[:, eb, :]
                        if eb % 2 == 0:
                            nc.vector.tensor_scalar(
                                dst, ps[eb][k][:], bec_sb[:, eb : eb + 1], None,
                                op0=mybir.AluOpType.add,
                            )
                        else:
                            nc.scalar.activation(
                                dst, ps[eb][k][:],
                                mybir.ActivationFunctionType.Identity,
                                bias=bec_sb[:, eb : eb + 1], scale=1.0,
                            )
                    nc.sync.dma_start(hT_d[:, :, ts(nt, NT)], stg[:])
    nc.compile()
    return nc


def _build_launch2():
    nc = bacc.Bacc(None, target_bir_lowering=False)
    hT_d = nc.dram_tensor("hT", [128, EB, NS], F32, kind="ExternalInput")
    v_d = nc.dram_tensor("v", [128, EB], F32, kind="ExternalInput")  # v/sqrt(E), blocked
    out_d = nc.dram_tensor("out2", [128, 8], F32, kind="ExternalOutput")

    import concourse.bass_isa as bass_isa

    with tile.TileContext(nc) as tc:
        with (
            tc.tile_pool(name="hpool", bufs=1) as hp,
            tc.tile_pool(name="spool", bufs=1) as sp,
            tc.tile_pool(name="wpool", bufs=3) as wbp,
            tc.tile_pool(name="dram", bufs=1, space="DRAM") as dp,
            tc.tile_pool(name="psum", bufs=4, space="PSUM") as pp,
        ):
            hT3 = hp.tile([128, EB, NS], F32)
            nc.sync.dma_start(hT3[:], hT_d[:])
            v_sb = sp.tile([128, EB], F32)
            nc.sync.dma_start(v_sb[:], v_d[:])

            s_row = sp.tile([1, NS], F32, tag="rowbuf")
            for nt in range(NTILES):
                ps_s = pp.tile([1, NT], F32, tag="ps")
                for eb in range(EB):
                    nc.tensor.matmul(
                        ps_s[:],
                        lhsT=v_sb[:, eb : eb + 1],
                        rhs=hT3[:, eb, ts(nt, NT)],
                        start=(eb == 0),
                        stop=(eb == EB - 1),
                    )
                nc.vector.tensor_copy(s_row[0:1, ts(nt, NT)], ps_s[:])

            # reshape row -> [128, 64] via DRAM bounce; n = p*64 + j
            row_dram = dp.tile([1, NS], F32)
            nc.sync.dma_start(row_dram[:], s_row[:])
            s2d = sp.tile([128, NS // 128], F32)
            nc.sync.dma_start(
                s2d[:], row_dram.rearrange("a (p j) -> p (a j)", p=128)
            )
            rmax = sp.tile([128, 1], F32)
            nc.vector.reduce_max(rmax[:], s2d[:], axis=mybir.AxisListType.X)
            mx = sp.tile([128, 1], F32)
            nc.gpsimd.partition_all_reduce(
                mx[:], rmax[:], channels=128, reduce_op=bass_isa.ReduceOp.max
            )
            negm = sp.tile([128, 1], F32)
            nc.vector.tensor_scalar_mul(negm[:], mx[:], -1.0)
            w2d = sp.tile([128, NS // 128], F32)
            ssum = sp.tile([128, 1], F32)
            nc.scalar.activation(
                w2d[:], s2d[:], mybir.ActivationFunctionType.Exp,
                bias=negm[:], scale=1.0, accum_out=ssum[:],
            )
            stot = sp.tile([128, 1], F32)
            nc.gpsimd.partition_all_reduce(
                stot[:], ssum[:], channels=128, reduce_op=bass_isa.ReduceOp.add
            )
            # back to a row via DRAM bounce (inverse reshape)
            wrow_dram = dp.tile([1, NS], F32)
            nc.sync.dma_start(
                wrow_dram.rearrange("a (p j) -> p (a j)", p=128), w2d[:]
            )
            w_row = sp.tile([1, NS], F32, tag="rowbuf")
            nc.sync.dma_start(w_row[:], wrow_dram[:])

            u_acc = sp.tile([128, EB], F32)
            nc.vector.memset(u_acc[:], 0.0)
            for nt in range(NTILES):
                w_bc = wbp.tile([128, NT], F32, tag="wbc")
                nc.gpsimd.partition_broadcast(
                    w_bc[:], w_row[0:1, ts(nt, NT)], channels=128
                )
                for eb in range(EB):
                    scr = wbp.tile([128, NT], F32, tag="scr")
                    red = wbp.tile([128, 1], F32, tag="red")
                    nc.vector.tensor_mul(scr[:], hT3[:, eb, ts(nt, NT)], w_bc[:])
                    nc.vector.reduce_sum(red[:], scr[:], axis=mybir.AxisListType.X)
                    nc.vector.tensor_add(
                        u_acc[:, eb : eb + 1], u_acc[:, eb : eb + 1], red[:]
                    )

            out_sb = sp.tile([128, 8], F32)
            nc.vector.memset(out_sb[:], 0.0)
            nc.vector.tensor_copy(out_sb[:, 0:EB], u_acc[:])
            nc.vector.tensor_copy(out_sb[:, 4:5], mx[:])
            nc.vector.tensor_copy(out_sb[:, 5:6], stot[:])
            nc.sync.dma_start(out_d[:], out_sb[:])
    nc.compile()
    return nc


def _blocked(v):
    """[E] -> [128, EB] with out[p, eb] = v[eb*128 + p]."""
    return np.ascontiguousarray(v.reshape(EB, 128).T)


def _unblocked(m):
    """[128, EB] -> [E] inverse of _blocked."""
    return np.ascontiguousarray(m.T.reshape(E))


def kernel(x, we, be, wi, bi, wq, bq, wk, bk, wb, bb):
    x = np.asarray(x, dtype=np.float32)
    we = np.ascontiguousarray(np.asarray(we, dtype=np.float32))
    be = np.asarray(be, dtype=np.float32)
    wi = np.asarray(wi, dtype=np.float32)
    bi = np.asarray(bi, dtype=np.float32)
    wq = np.asarray(wq, dtype=np.float32)
    bq = np.asarray(bq, dtype=np.float32)
    wk = np.asarray(wk, dtype=np.float32)
    bk = np.asarray(bk, dtype=np.float32)
    wb = np.asarray(wb, dtype=np.float32)
    bb = np.asarray(bb, dtype=np.float32)

    if "l1" not in _cache:
        _cache["l1"] = _build_launch1()
    if "l2" not in _cache:
        _cache["l2"] = _build_launch2()

    bec = _blocked(be)

    # per-core shards: core c -> (bag c//2, half c%2)
    in_maps1 = []
    for c in range(NCORES):
        b, h = divmod(c, 2)
        xs = x[b, h * NS : (h + 1) * NS, :]          # [NS, D]
        xt = np.ascontiguousarray(xs.T)              # [D, NS]
        in_maps1.append({"xt": xt, "we": we, "bec": bec})

    res1 = run_bass_kernel_spmd(
        _cache["l1"], in_maps1, core_ids=list(range(NCORES))
    ).results
    hT = [r["hT"] for r in res1]  # each [128, EB, NS]

    # ---- host glue: instance scores -> critical instance -> q, v ----
    # h_c as [NS, E]: h[n, eb*128+p] = hT[p, eb, n]
    h_flat = [
        np.ascontiguousarray(t.transpose(2, 1, 0).reshape(NS, E)) for t in hT
    ]
    scale = np.float32(E) ** 0.5
    v_cols = [None] * NCORES
    crit = [None] * B
    for b in range(B):
        c0, c1 = 2 * b, 2 * b + 1
        best = None
        for c in (c0, c1):
            logits = h_flat[c] @ wi + bi            # [NS, C]
            s = logits.max(axis=1)                   # [NS]
            i = int(s.argmax())
            if best is None or s[i] > best[0]:
                best = (s[i], c, i)
        _, cw, iw = best
        cr = h_flat[cw][iw]                          # [E]
        crit[b] = cr
        q = cr @ wq + bq                             # [E]
        v = (wk @ q) / scale                         # [E]
        vc = _blocked(v.astype(np.float32))
        v_cols[c0] = vc
        v_cols[c1] = vc

    in_maps2 = [
        {"hT": np.ascontiguousarray(hT[c]), "v": v_cols[c]} for c in range(NCORES)
    ]
    res2 = run_bass_kernel_spmd(
        _cache["l2"], in_maps2, core_ids=list(range(NCORES))
    ).results

    # ---- host combine: online softmax across the two halves of each bag ----
    out = np.zeros((B, C), dtype=np.float32)
    for b in range(B):
        parts = []
        for c in (2 * b, 2 * b + 1):
            o = res2[c]["out2"]
            U = _unblocked(o[:, 0:EB])               # [E]
            m = float(o[0, 4])
            S = float(o[0, 5])
            parts.append((m, S, U))
        m_star = max(p[0] for p in parts)
        S_tot = 0.0
        U_tot = np.zeros(E, dtype=np.float64)
        for m, S, U in parts:
            f = np.exp(m - m_star)
            S_tot += S * f
            U_tot += U.astype(np.float64) * f
        attn_bag = (U_tot / S_tot).astype(np.float32)
        fused = np.concatenate([crit[b], attn_bag])  # [2E]
        out[b] = fused @ wb + bb
    return out
